# revision 26
# baseline (speedup 1.0000x reference)
"""KAN B-spline activation kernel for Trainium2 (8 NeuronCores, data-parallel on batch).

Math: for the uniform grid (spacing h, g3[k] = knots[8+k]) and x in [0,1),
only cubic bases b3[8..18] are nonzero, and each is the cardinal B-spline
kernel K at the distance to its center:
  A2[m]  = x - g3[m+2],  sig[m] = |A2[m]|                       m = 0..11
  r2n = min(sig-2h, 0) = -relu(2h-sig);  r1n = min(sig-h, 0)
  T[m] = 4*r1n^3 - r2n^3 = h^3*(relu(2-s)^3 - 4*relu(1-s)^3) = 6h^3*b3[m+8]
  out[b,o,i] = sum_m T[b,i,m] * coef[o,i,8+m] / (6 h^3)     (T[11] == 0)
Validated vs reference: 1.5e-3 rel err with the full fp16 chain.

Device schedule (per core, 128 batch rows in partitions):
  - no grid tensor: constants baked into tensor_scalar immediates.
  - whole chain in fp16 on DVE, one 32-input half at a time: tensor_scalar
    ops (single-src) hit the fast DVE modes, tensor_tensor (cubes) gets 2x;
    all tiles 12-wide so rows are 4B-aligned with even counts.
  - 8 PE transposes of 128-col groups -> basesT ((input,knot) partitions,
    batch free) fp16, one PSUM bank tile per chunk (avoids transpose/copy
    WAR serialization), copied out per chunk so matmuls start early;
    8 fp16 matmuls (K=128, N=512) vs a block-diagonal fp16 host-built rhs.
  - PSUM->SBUF output copies mostly on the scalar engine (vector is busy
    with the second half); 5 rolling output DMAs, the last two single-group
    so the tail transfer is small.
    Host upcasts fp16 and un-permutes (b, g, i_l, o) -> (b, o, i).
"""

import numpy as np
from contextlib import ExitStack

import concourse.bass as bass
import concourse.tile as tile
from concourse import bacc, mybir
from concourse.bass_utils import run_bass_kernel_spmd
from concourse.masks import make_identity

N_CORES = 8
B_TOT, IN_DIM, OUT_DIM = 1024, 64, 64
BPC = B_TOT // N_CORES          # 128 batch rows per core
K16 = 16                        # padded knot window per input
NG = 8                          # groups of 8 inputs
F32 = mybir.dt.float32
F16 = mybir.dt.float16

_CACHE = {}


def _build_nc(g3_2, h):
    AL = mybir.AluOpType
    AF = mybir.ActivationFunctionType
    nc = bacc.Bacc("TRN2", target_bir_lowering=False, debug=False,
                   num_devices=N_CORES)
    x_d = nc.dram_tensor("x_in", [BPC, IN_DIM], F16, kind="ExternalInput").ap()
    rhs_d = nc.dram_tensor("rhs_in", [128, NG * 512], F16,
                           kind="ExternalInput").ap()
    out_d = nc.dram_tensor("out", [BPC, NG * 512], F16,
                           kind="ExternalOutput").ap()

    with tile.TileContext(nc) as tc, ExitStack() as ctx:
        pool = ctx.enter_context(tc.tile_pool(name="main", bufs=1))
        psT = ctx.enter_context(tc.tile_pool(name="psT", bufs=1, space="PSUM"))
        psO = ctx.enter_context(tc.tile_pool(name="psO", bufs=4, space="PSUM"))

        x_sb = pool.tile([BPC, IN_DIM], F16)
        rhs_sb = pool.tile([128, NG * 512], F16)
        nc.sync.dma_start(out=x_sb[:], in_=x_d)
        nc.sync.dma_start(out=rhs_sb[:, :4 * 512], in_=rhs_d[:, :4 * 512])
        nc.sync.dma_start(out=rhs_sb[:, 4 * 512:], in_=rhs_d[:, 4 * 512:])

        ident = pool.tile([128, 128], F16)
        make_identity(nc, ident)

        A2 = pool.tile([BPC, IN_DIM, 12], F16)
        sig = pool.tile([BPC, IN_DIM, 12], F16)
        r2n = pool.tile([BPC, IN_DIM, 12], F16)
        r1n = pool.tile([BPC, IN_DIM, 12], F16)
        sq2 = pool.tile([BPC, IN_DIM, 12], F16)
        sq1 = pool.tile([BPC, IN_DIM, 12], F16)
        cu2 = pool.tile([BPC, IN_DIM, 12], F16)
        cu1 = pool.tile([BPC, IN_DIM, 12], F16)
        c4 = pool.tile([BPC, IN_DIM, 12], F16)
        B3h = pool.tile([BPC, IN_DIM, K16], F16)

        # pad knots 12..15 feed the transpose -> real coef columns: keep 0
        nc.gpsimd.memset(B3h[:, :, 12:16], 0.0)

        V = nc.vector
        S = nc.scalar
        HW = IN_DIM // 2
        xv = x_sb[:].rearrange("p (i k) -> p i k", k=1)

        # whole chain on DVE, one half at a time so PE transposes start early
        def half_chain(s):
            # A2[k] = x - g3[2+k]
            V.tensor_scalar_sub(A2[:, s, 0:1], xv[:, s, :], g3_2)
            V.tensor_scalar_sub(A2[:, s, 1:2], xv[:, s, :], g3_2 + h)
            V.tensor_scalar_sub(A2[:, s, 2:4], A2[:, s, 0:2], 2.0 * h)
            V.tensor_scalar_sub(A2[:, s, 4:8], A2[:, s, 0:4], 4.0 * h)
            V.tensor_scalar_sub(A2[:, s, 8:12], A2[:, s, 0:4], 8.0 * h)
            # sig = |A2|; negative relu legs r_n = min(sig-c, 0) = -relu(c-sig)
            V.tensor_scalar_mul(c4[:, s, :], A2[:, s, :], -1.0)   # scratch
            V.tensor_max(sig[:, s, :], A2[:, s, :], c4[:, s, :])
            V.tensor_scalar(r2n[:, s, :], sig[:, s, :], 2.0 * h, 0.0,
                            AL.subtract, AL.min)
            V.tensor_scalar(r1n[:, s, :], sig[:, s, :], 1.0 * h, 0.0,
                            AL.subtract, AL.min)
            V.tensor_mul(sq2[:, s, :], r2n[:, s, :], r2n[:, s, :])
            V.tensor_mul(sq1[:, s, :], r1n[:, s, :], r1n[:, s, :])
            V.tensor_mul(cu2[:, s, :], sq2[:, s, :], r2n[:, s, :])
            V.tensor_mul(cu1[:, s, :], sq1[:, s, :], r1n[:, s, :])
            V.tensor_scalar_mul(c4[:, s, :], cu1[:, s, :], 4.0)
            # T = 4*cu1n - cu2n = r2p^3 - 4*r1p^3
            V.tensor_sub(B3h[:, s, 0:12], c4[:, s, :], cu2[:, s, :])

        half_chain(slice(0, HW))
        half_chain(slice(HW, IN_DIM))

        basesT = pool.tile([128, NG * 128], F16)
        og = pool.tile([BPC, NG * 512], F16)
        # PSUM allocation is bank-granular: 4 chunk tiles, reused per half
        psc = [psT.tile([128, 128], F16, name=f"psc{g}") for g in range(4)]

        B3f = B3h[:].rearrange("p i k -> p (i k)")

        def transp(g):
            nc.tensor.transpose(out=psc[g % 4][:],
                                in_=B3f[:, g * 128:(g + 1) * 128],
                                identity=ident[:])

        def mm(g):
            ps_o = psO.tile([128, 512], F32)
            nc.tensor.matmul(out=ps_o[:],
                             lhsT=basesT[:, g * 128:(g + 1) * 128],
                             rhs=rhs_sb[:, g * 512:(g + 1) * 512],
                             start=True, stop=True)
            return ps_o

        for g in range(4):
            transp(g)
            # chunk copies on Act: vector is busy with the second half
            S.copy(basesT[:, g * 128:(g + 1) * 128], psc[g % 4][:])
        pso_l = [mm(0), mm(1), mm(2), mm(3)]
        for g in range(4, NG):
            transp(g)
            nc.vector.tensor_copy(basesT[:, g * 128:(g + 1) * 128],
                                  psc[g % 4][:])
        # og copies: Act takes g0-5 (vector busy until T_H1 + bases copies);
        # vector takes g6 and half of g7
        for g in range(4):
            S.copy(og[:, g * 512:(g + 1) * 512], pso_l[g][:])
        for g in range(4, NG):
            ps_o = mm(g)
            if g < 6:
                S.copy(og[:, g * 512:(g + 1) * 512], ps_o[:])
            else:
                nc.vector.tensor_copy(og[:, g * 512:(g + 1) * 512], ps_o[:])

        for lo, hi in ((0, 2), (2, 4), (4, 6), (6, 7), (7, 8)):
            nc.sync.dma_start(out=out_d[:, lo * 512:hi * 512],
                              in_=og[:, lo * 512:hi * 512])

    nc.compile()
    return nc


def _host_inputs(x, coef, grid):
    x = np.asarray(x, dtype=np.float32).astype(np.float16)
    coef = np.asarray(coef, dtype=np.float32)
    knots = np.asarray(grid, dtype=np.float32)[0, 0, :]          # (23,)
    h = float(knots[1] - knots[0])
    g3_2 = float(knots[10])

    cf = coef[:, :, 8:19].astype(np.float64) / (6.0 * h**3)      # (o, i, 11)
    # block-diagonal rhs per group: rows (i_l,m) x cols (i_l', o), K=128, N=512
    rhs = np.zeros((128, NG * 512), dtype=np.float16)
    for i_l in range(8):
        for g in range(NG):
            i = g * 8 + i_l
            rhs[i_l * 16:i_l * 16 + 11,
                g * 512 + i_l * 64:g * 512 + i_l * 64 + 64] = (
                    cf[:, i, :].T.astype(np.float16))
    return x, rhs, g3_2, h


def _execute(x, coef, grid, trace=False, **spmd_kwargs):
    xf, rhs, g3_2, h = _host_inputs(x, coef, grid)
    if "nc" not in _CACHE:
        _CACHE["nc"] = _build_nc(g3_2, h)
    nc = _CACHE["nc"]
    in_maps = [{"x_in": np.ascontiguousarray(xf[c * BPC:(c + 1) * BPC]),
                "rhs_in": rhs} for c in range(N_CORES)]
    res = run_bass_kernel_spmd(nc, in_maps, list(range(N_CORES)),
                               trace=trace, **spmd_kwargs)
    full = np.empty((B_TOT, OUT_DIM, IN_DIM), dtype=np.float32)
    for c in range(N_CORES):
        t = res.results[c]["out"].astype(np.float32)
        t = t.reshape(BPC, NG, 8, 64)                            # (b, g, i_l, o)
        full[c * BPC:(c + 1) * BPC] = (
            t.transpose(0, 3, 1, 2).reshape(BPC, OUT_DIM, IN_DIM))
    return full, res


def kernel(x, coef, grid):
    out, _ = _execute(x, coef, grid, trace=False)
    return out


# revision 27
# speedup vs baseline: 1.0112x; 1.0112x over previous
"""KAN B-spline activation kernel for Trainium2 (8 NeuronCores, data-parallel on batch).

Math: for the uniform grid (spacing h, g3[k] = knots[8+k]) and x in [0,1),
only cubic bases b3[8..18] are nonzero, and each is the cardinal B-spline
kernel K at the distance to its center:
  A2[m]  = x - g3[m+2],  sig[m] = |A2[m]|                       m = 0..11
  r2n = min(sig-2h, 0) = -relu(2h-sig);  r1n = min(sig-h, 0)
  T[m] = 4*r1n^3 - r2n^3 = h^3*(relu(2-s)^3 - 4*relu(1-s)^3) = 6h^3*b3[m+8]
  out[b,o,i] = sum_m T[b,i,m] * coef[o,i,8+m] / (6 h^3)     (T[11] == 0)
Validated vs reference: 1.5e-3 rel err with the full fp16 chain.

Device schedule (per core, 128 batch rows in partitions):
  - no grid tensor: constants baked into tensor_scalar immediates.
  - whole chain in fp16 on DVE, one 32-input half at a time: tensor_scalar
    ops (single-src) hit the fast DVE modes, tensor_tensor (cubes) gets 2x;
    all tiles 12-wide so rows are 4B-aligned with even counts.
  - 8 PE transposes of 128-col groups -> basesT ((input,knot) partitions,
    batch free) fp16, one PSUM bank tile per chunk (avoids transpose/copy
    WAR serialization), copied out per chunk so matmuls start early;
    8 fp16 matmuls (K=128, N=512) vs a block-diagonal fp16 host-built rhs.
  - PSUM->SBUF output copies mostly on the scalar engine (vector is busy
    with the second half); 5 rolling output DMAs, the last two single-group
    so the tail transfer is small.
    Host upcasts fp16 and un-permutes (b, g, i_l, o) -> (b, o, i).
"""

import numpy as np
from contextlib import ExitStack

import concourse.bass as bass
import concourse.tile as tile
from concourse import bacc, mybir
from concourse.bass_utils import run_bass_kernel_spmd
from concourse.masks import make_identity

N_CORES = 8
B_TOT, IN_DIM, OUT_DIM = 1024, 64, 64
BPC = B_TOT // N_CORES          # 128 batch rows per core
K16 = 16                        # padded knot window per input
NG = 8                          # groups of 8 inputs
F32 = mybir.dt.float32
F16 = mybir.dt.float16

_CACHE = {}


def _build_nc(g3_2, h):
    AL = mybir.AluOpType
    AF = mybir.ActivationFunctionType
    nc = bacc.Bacc("TRN2", target_bir_lowering=False, debug=False,
                   num_devices=N_CORES)
    x_d = nc.dram_tensor("x_in", [BPC, IN_DIM], F16, kind="ExternalInput").ap()
    rhs_d = nc.dram_tensor("rhs_in", [128, NG * 512], F16,
                           kind="ExternalInput").ap()
    out_d = nc.dram_tensor("out", [BPC, NG * 512], F16,
                           kind="ExternalOutput").ap()

    with tile.TileContext(nc) as tc, ExitStack() as ctx:
        pool = ctx.enter_context(tc.tile_pool(name="main", bufs=1))
        psT = ctx.enter_context(tc.tile_pool(name="psT", bufs=1, space="PSUM"))
        psO = ctx.enter_context(tc.tile_pool(name="psO", bufs=4, space="PSUM"))

        x_sb = pool.tile([BPC, IN_DIM], F16)
        rhs_sb = pool.tile([128, NG * 512], F16)
        nc.sync.dma_start(out=x_sb[:], in_=x_d)
        nc.sync.dma_start(out=rhs_sb[:, :4 * 512], in_=rhs_d[:, :4 * 512])
        nc.sync.dma_start(out=rhs_sb[:, 4 * 512:], in_=rhs_d[:, 4 * 512:])

        ident = pool.tile([128, 128], F16)
        make_identity(nc, ident)

        A2 = pool.tile([BPC, IN_DIM, 12], F16)
        sig = pool.tile([BPC, IN_DIM, 12], F16)
        r2n = pool.tile([BPC, IN_DIM, 12], F16)
        r1n = pool.tile([BPC, IN_DIM, 12], F16)
        sq2 = pool.tile([BPC, IN_DIM, 12], F16)
        sq1 = pool.tile([BPC, IN_DIM, 12], F16)
        cu2 = pool.tile([BPC, IN_DIM, 12], F16)
        cu1 = pool.tile([BPC, IN_DIM, 12], F16)
        c4 = pool.tile([BPC, IN_DIM, 12], F16)
        B3h = pool.tile([BPC, IN_DIM, K16], F16)

        # pad knots 12..15 feed the transpose -> real coef columns: keep 0
        nc.gpsimd.memset(B3h[:, :, 12:16], 0.0)

        V = nc.vector
        S = nc.scalar
        HW = IN_DIM // 2
        xv = x_sb[:].rearrange("p (i k) -> p i k", k=1)

        # whole chain on DVE, one half at a time so PE transposes start early
        def half_chain(s):
            # A2[k] = x - g3[2+k]
            V.tensor_scalar_sub(A2[:, s, 0:1], xv[:, s, :], g3_2)
            V.tensor_scalar_sub(A2[:, s, 1:2], xv[:, s, :], g3_2 + h)
            V.tensor_scalar_sub(A2[:, s, 2:4], A2[:, s, 0:2], 2.0 * h)
            V.tensor_scalar_sub(A2[:, s, 4:8], A2[:, s, 0:4], 4.0 * h)
            V.tensor_scalar_sub(A2[:, s, 8:12], A2[:, s, 0:4], 8.0 * h)
            # sig = |A2|; negative relu legs r_n = min(sig-c, 0) = -relu(c-sig)
            V.tensor_scalar_mul(c4[:, s, :], A2[:, s, :], -1.0)   # scratch
            V.tensor_max(sig[:, s, :], A2[:, s, :], c4[:, s, :])
            V.tensor_scalar(r2n[:, s, :], sig[:, s, :], 2.0 * h, 0.0,
                            AL.subtract, AL.min)
            V.tensor_scalar(r1n[:, s, :], sig[:, s, :], 1.0 * h, 0.0,
                            AL.subtract, AL.min)
            V.tensor_mul(sq2[:, s, :], r2n[:, s, :], r2n[:, s, :])
            V.tensor_mul(sq1[:, s, :], r1n[:, s, :], r1n[:, s, :])
            V.tensor_mul(cu2[:, s, :], sq2[:, s, :], r2n[:, s, :])
            V.tensor_mul(cu1[:, s, :], sq1[:, s, :], r1n[:, s, :])
            V.tensor_scalar_mul(c4[:, s, :], cu1[:, s, :], 4.0)
            # T = 4*cu1n - cu2n = r2p^3 - 4*r1p^3
            V.tensor_sub(B3h[:, s, 0:12], c4[:, s, :], cu2[:, s, :])

        half_chain(slice(0, HW))
        half_chain(slice(HW, IN_DIM))

        basesT = pool.tile([128, NG * 128], F16)
        og = pool.tile([BPC, NG * 512], F16)
        # PSUM allocation is bank-granular: 4 chunk tiles, reused per half
        psc = [psT.tile([128, 128], F16, name=f"psc{g}") for g in range(4)]

        B3f = B3h[:].rearrange("p i k -> p (i k)")

        def transp(g):
            nc.tensor.transpose(out=psc[g % 4][:],
                                in_=B3f[:, g * 128:(g + 1) * 128],
                                identity=ident[:])

        def mm(g):
            ps_o = psO.tile([128, 512], F32)
            nc.tensor.matmul(out=ps_o[:],
                             lhsT=basesT[:, g * 128:(g + 1) * 128],
                             rhs=rhs_sb[:, g * 512:(g + 1) * 512],
                             start=True, stop=True)
            return ps_o

        for g in range(4):
            transp(g)
            # chunk copies on Act: vector is busy with the second half
            S.copy(basesT[:, g * 128:(g + 1) * 128], psc[g % 4][:])
        pso_l = [mm(0), mm(1), mm(2), mm(3)]
        for g in range(4, NG):
            transp(g)
            nc.vector.tensor_copy(basesT[:, g * 128:(g + 1) * 128],
                                  psc[g % 4][:])
        # og copies: Act takes g0-5 (vector busy until T_H1 + bases copies);
        # vector takes g6 and half of g7
        for g in range(4):
            S.copy(og[:, g * 512:(g + 1) * 512], pso_l[g][:])
        for g in range(4, NG):
            ps_o = mm(g)
            if g < 6:
                S.copy(og[:, g * 512:(g + 1) * 512], ps_o[:])
            elif g == 6:
                nc.vector.tensor_copy(og[:, g * 512:(g + 1) * 512], ps_o[:])
            else:
                nc.vector.tensor_copy(og[:, g * 512:g * 512 + 256],
                                      ps_o[:, 0:256])
                S.copy(og[:, g * 512 + 256:(g + 1) * 512], ps_o[:, 256:512])

        for lo, hi in ((0, 2), (2, 4), (4, 6), (6, 7), (7, 8)):
            nc.sync.dma_start(out=out_d[:, lo * 512:hi * 512],
                              in_=og[:, lo * 512:hi * 512])

    nc.compile()
    return nc


def _host_inputs(x, coef, grid):
    x = np.asarray(x, dtype=np.float32).astype(np.float16)
    coef = np.asarray(coef, dtype=np.float32)
    knots = np.asarray(grid, dtype=np.float32)[0, 0, :]          # (23,)
    h = float(knots[1] - knots[0])
    g3_2 = float(knots[10])

    cf = coef[:, :, 8:19].astype(np.float64) / (6.0 * h**3)      # (o, i, 11)
    # block-diagonal rhs per group: rows (i_l,m) x cols (i_l', o), K=128, N=512
    rhs = np.zeros((128, NG * 512), dtype=np.float16)
    for i_l in range(8):
        for g in range(NG):
            i = g * 8 + i_l
            rhs[i_l * 16:i_l * 16 + 11,
                g * 512 + i_l * 64:g * 512 + i_l * 64 + 64] = (
                    cf[:, i, :].T.astype(np.float16))
    return x, rhs, g3_2, h


def _execute(x, coef, grid, trace=False, **spmd_kwargs):
    xf, rhs, g3_2, h = _host_inputs(x, coef, grid)
    if "nc" not in _CACHE:
        _CACHE["nc"] = _build_nc(g3_2, h)
    nc = _CACHE["nc"]
    in_maps = [{"x_in": np.ascontiguousarray(xf[c * BPC:(c + 1) * BPC]),
                "rhs_in": rhs} for c in range(N_CORES)]
    res = run_bass_kernel_spmd(nc, in_maps, list(range(N_CORES)),
                               trace=trace, **spmd_kwargs)
    full = np.empty((B_TOT, OUT_DIM, IN_DIM), dtype=np.float32)
    for c in range(N_CORES):
        t = res.results[c]["out"].astype(np.float32)
        t = t.reshape(BPC, NG, 8, 64)                            # (b, g, i_l, o)
        full[c * BPC:(c + 1) * BPC] = (
            t.transpose(0, 3, 1, 2).reshape(BPC, OUT_DIM, IN_DIM))
    return full, res


def kernel(x, coef, grid):
    out, _ = _execute(x, coef, grid, trace=False)
    return out
